# revision 25
# baseline (speedup 1.0000x reference)
"""Trainium2 Bass kernel for MultiHeadAttention + residual + LayerNorm.

Problem (fixed shapes):
  B=2, S=2048, H=1024, NH=16, HD=64
  out = LayerNorm(query + (softmax(scale * (query Wq^T)(key Wk^T)^T) (value Wv^T)) Wo^T + bo)

Sharding: 8 cores, data-parallel over (batch, query-token-shard):
  core c -> batch b = c // 4, query rows [ (c%4)*512, (c%4+1)*512 ) of batch b.
Each core recomputes K/V projections for its batch (replicated within the 4
cores of a batch) so no cross-core communication is needed; the host only
slices inputs and concatenates the 8 output shards.

Per-core dataflow (all matmuls on the PE array, fp32r for projections,
bf16 for attention, fp32 accumulation in PSUM):
  P3: qT[j, t]  = Wq^T as lhsT,  xqT as moving     (transposed layout)
  P1: kT[j, t]  = same, over all 2048 keys
  P2: v[t, d]   = xvT as lhsT,   Wv^T as moving    (normal layout), with
      constant-1.0 columns appended on both sides of v so that the ctx
      matmul simultaneously produces the softmax denominator rows.
  P4: per head h: logitsT[k, q] = kT_h^T qT_h ; probsT = exp(scale*logitsT)
      ctxT_aug = [v_h | ones]^T probsT  -> ctx rows + denominator rows
      ctxT_h = ctx rows * (1/denom) broadcast via a K=1 matmul
  P5: projected[t, o] = ctxT as lhsT, Wo^T as moving; + query residual + bo;
      LayerNorm over H via bn_stats/bn_aggr; DMA out.
"""

from contextlib import ExitStack

import numpy as np

import concourse.bass as bass
import concourse.mybir as mybir
import concourse.tile as tile
from concourse import bacc

B, S, H = 2, 2048, 1024
NH, HD = 16, 64
LN_EPS = 1e-5
N_CORES = 8
SHARD = B * S // N_CORES  # 512 query rows per core

F32 = mybir.dt.float32
F32R = mybir.dt.float32r
BF16 = mybir.dt.bfloat16
F16 = mybir.dt.float16

# v tile column layout: 8 head-pair groups of [v_even(64) | ones(64) | v_odd(64)]
# so that every head's [v|ones] (even) / [ones|v] (odd) stationary operand is
# one contiguous 128-column slice (walrus allows only one free dim on matmul APs).
VCOLS = (NH // 2) * 192  # 1536


def _bcast_row(dram_ap_row, parts=128):
    """Partition-broadcast a [1, N] DRAM row to `parts` partitions for DMA."""
    return bass.AP(
        tensor=dram_ap_row.tensor,
        offset=dram_ap_row.offset,
        ap=[[0, parts]] + list(dram_ap_row.ap)[-1:],
    )


def _v_head_ap(v_sb, kt, h):
    """Contiguous 128-col slice [v_h | ones] (even h) / [ones | v_h] (odd h).

    For even h the ones block comes second -> PE output partitions 0-63 hold
    ctx, 64-127 hold the denominator; for odd h it is reversed. This parity
    matches where ctxT_sb wants the rows, keeping vector ops partition-aligned.
    """
    start = (h // 2) * 192 + (64 if h % 2 else 0)
    ap = v_sb[:, kt, start : start + 128]
    assert ap.free_size() == 128 and ap.partition_size() == 128
    return ap


def build_kernel(dbg: bool = False, upto: int = 5, reps: int = 1, xbufs: int = 5, prbufs: int = 3, wbufs: int = 4):
    nc = bacc.Bacc("TRN2", target_bir_lowering=False, debug=False)

    xqT = nc.dram_tensor("xqT", [H, SHARD], BF16, kind="ExternalInput").ap()
    xkT = nc.dram_tensor("xkT", [H, S], BF16, kind="ExternalInput").ap()
    xvT = nc.dram_tensor("xvT", [H, S], BF16, kind="ExternalInput").ap()
    wqT = nc.dram_tensor("wqT", [H, H], BF16, kind="ExternalInput").ap()
    wkT = nc.dram_tensor("wkT", [H, H], BF16, kind="ExternalInput").ap()
    wvT = nc.dram_tensor("wvT", [H, H], BF16, kind="ExternalInput").ap()
    woT = nc.dram_tensor("woT", [H, H], F32R, kind="ExternalInput").ap()
    qres = nc.dram_tensor("qres", [SHARD, H], F32, kind="ExternalInput").ap()
    gbb = nc.dram_tensor("gbb", [2, H], F32, kind="ExternalInput").ap()
    out = nc.dram_tensor("out", [SHARD, H], F32, kind="ExternalOutput").ap()
    dbg_t = {}
    if dbg:
        dbg_t["qT"] = nc.dram_tensor("dbg_qT", [128, H // 128, SHARD], BF16, kind="ExternalOutput").ap()
        dbg_t["v"] = nc.dram_tensor("dbg_v", [128, S // 128, VCOLS], BF16, kind="ExternalOutput").ap()
        dbg_t["ctxT"] = nc.dram_tensor("dbg_ctxT", [128, H // 128, SHARD], F32, kind="ExternalOutput").ap()
        dbg_t["probs0"] = nc.dram_tensor("dbg_probs0", [128, 512], BF16, kind="ExternalOutput").ap()
        dbg_t["pc0"] = nc.dram_tensor("dbg_pc0", [128, 512], F32, kind="ExternalOutput").ap()

    # [feature-tile p, feature-group a, token t] views for DMA
    xqT_r = xqT.rearrange("(a p) t -> p a t", p=128)
    xkT_r = xkT.rearrange("(a p) t -> p a t", p=128)
    xvT_r = xvT.rearrange("(a p) t -> p a t", p=128)
    w_r = {
        "wq": wqT.rearrange("(a p) j -> p a j", p=128),
        "wk": wkT.rearrange("(a p) j -> p a j", p=128),
        "wv": wvT.rearrange("(a p) j -> p a j", p=128),
        "wo": woT.rearrange("(a p) j -> p a j", p=128),
    }
    qres_r = qres.rearrange("(a p) j -> p a j", p=128)

    KCH = S // 512  # 4 key-token chunks
    QCH = SHARD // 512  # 1 query chunk

    with ExitStack() as ctx:
        ctx.enter_context(nc.allow_low_precision(reason="bf16 attention + f32r matmuls by design"))
        tc = ctx.enter_context(tile.TileContext(nc))
        wpool = ctx.enter_context(tc.tile_pool(name="wpool", bufs=wbufs))
        xpool = ctx.enter_context(tc.tile_pool(name="xpool", bufs=xbufs))
        kT_pool = ctx.enter_context(tc.tile_pool(name="kT", bufs=3))
        v_pool = ctx.enter_context(tc.tile_pool(name="vv", bufs=1))
        qT_pool = ctx.enter_context(tc.tile_pool(name="qT", bufs=1))
        ctx_pool = ctx.enter_context(tc.tile_pool(name="ctxT", bufs=1))
        probs_pool = ctx.enter_context(tc.tile_pool(name="probs", bufs=prbufs))
        small_pool = ctx.enter_context(tc.tile_pool(name="smalls", bufs=2))
        const_pool = ctx.enter_context(tc.tile_pool(name="consts", bufs=1))
        qres_pool = ctx.enter_context(tc.tile_pool(name="qresp", bufs=2))
        out_pool = ctx.enter_context(tc.tile_pool(name="outp", bufs=2))
        stat_pool = ctx.enter_context(tc.tile_pool(name="stats", bufs=2))
        dram_pool = ctx.enter_context(tc.tile_pool(name="drp", bufs=4, space="DRAM"))
        pp = ctx.enter_context(tc.tile_pool(name="pp", bufs=2, space="PSUM"))
        pl = ctx.enter_context(tc.tile_pool(name="pl", bufs=2, space="PSUM"))
        pc = ctx.enter_context(tc.tile_pool(name="pc", bufs=2, space="PSUM"))
        rep_ctx = tc.For_i(0, reps, 1) if reps > 1 else None
        if rep_ctx is not None:
            ctx.enter_context(rep_ctx)
        if True:
            # ---- constants ----
            eps_t = const_pool.tile([128, 1], F32)
            nc.vector.memset(eps_t, LN_EPS)
            ga_b = const_pool.tile([128, H], F32)
            be_b = const_pool.tile([128, H], F32)
            for i, t in enumerate((ga_b, be_b)):
                nc.sync.dma_start(out=t, in_=_bcast_row(gbb[i : i + 1, :]))

            # ---- persistent activations ----
            v_sb = v_pool.tile([128, S // 128, VCOLS], BF16)  # v[t, d] + ones
            qT_sb = qT_pool.tile([128, H // 128, SHARD], BF16)  # qT[j, t]
            ctxT_sb = ctx_pool.tile([128, H // 128, SHARD], F32R)  # ctxT[i, t]

            def load_w_halves(name):
                # two bf16 tiles of 4 feature-groups each (8KB/partition slots)
                h0 = wpool.tile([128, 4, H], BF16, tag="w")
                h1 = wpool.tile([128, 4, H], BF16, tag="w")
                nc.sync.dma_start(out=h0, in_=w_r[name][:, 0:4, :])
                nc.sync.dma_start(out=h1, in_=w_r[name][:, 4:8, :])
                return (h0, h1)

            def load_w_quarters(name):
                # four f32r tiles of 2 feature-groups each (8KB/partition slots)
                qs = []
                for i in range(4):
                    q = wpool.tile([128, 2, H], F32R, tag="w")
                    nc.sync.dma_start(out=q, in_=w_r[name][:, 2 * i : 2 * i + 2, :])
                    qs.append(q)
                return qs

            def load_x_chunk(x_r, tt):
                t = xpool.tile([128, 8, 512], BF16, tag="x")
                nc.sync.dma_start(out=t, in_=x_r[:, :, tt * 512 : (tt + 1) * 512])
                return (t, t)

            # ---- P3: q projection (transposed layout, upfront) ----
            wq = load_w_halves("wq")
            for tt in range(QCH):
                xq = load_x_chunk(xqT_r, tt)
                for aj in range(8):
                    pp_t = pp.tile([128, 512], F32, tag="pp")
                    for ai in range(8):
                        nc.tensor.matmul(
                            pp_t,
                            lhsT=(wq[ai // 4][:, ai % 4, aj * 128 : (aj + 1) * 128]),
                            rhs=(xq[0][:, ai, :]),
                            start=(ai == 0),
                            stop=(ai == 7),
                        )
                    nc.vector.tensor_copy(
                        out=qT_sb[:, aj, tt * 512 : (tt + 1) * 512], in_=pp_t
                    )

            # ---- P2: v projection (normal layout, with ones columns) ----
            wv = load_w_halves("wv")
            for tt in range(KCH):
                xv = load_x_chunk(xvT_r, tt)
                for tloc in range(4):
                    at = tt * 4 + tloc
                    base = v_sb[:, at, :]
                    ones_ap = bass.AP(
                        tensor=base.tensor,
                        offset=base.offset + 64,
                        ap=[list(base.ap)[0], [192, NH // 2], [1, 64]],
                    )
                    nc.vector.memset(ones_ap, 1.0)
                    trow = slice(tloc * 128, (tloc + 1) * 128)
                    for dt in range(2):
                        pp_t = pp.tile([128, 512], F32, tag="pp")
                        for ai in range(8):
                            nc.tensor.matmul(
                                pp_t,
                                lhsT=(xv[0][:, ai, trow]),
                                rhs=(
                                    wv[ai // 4][:, ai % 4, dt * 512 : (dt + 1) * 512]
                                ),
                                start=(ai == 0),
                                stop=(ai == 7),
                            )
                        v_dst = bass.AP(
                            tensor=base.tensor,
                            offset=base.offset + dt * 4 * 192,
                            ap=[list(base.ap)[0], [192, 4], [128, 2], [1, 64]],
                        )
                        nc.vector.tensor_copy(out=v_dst, in_=pp_t)

            # ---- prefetch output-projection weight + residual ----
            wk = load_w_halves("wk")
            wo = load_w_quarters("wo") if upto >= 5 else None

            # ---- P1+P4 interleaved per head pair ----
            # xk stays resident (4 chunks); kproj for feature block m (heads
            # 2m, 2m+1) writes a small rotating kT tile, then both heads'
            # logits/exp/ctx run while the next pair's kproj proceeds -> the
            # ACT exp stream starts ~1/8 into the k projection.
            NKT = S // 128
            xk_res = [load_x_chunk(xkT_r, tt) for tt in range(KCH)]
            for m in range(NH // 2):
                kT_sb = kT_pool.tile([128, S], BF16, tag="kt", name=f"kT_{m}")
                for tt in range(KCH):
                    pp_t = pp.tile([128, 512], F32, tag="pp")
                    for ai in range(8):
                        nc.tensor.matmul(
                            pp_t,
                            lhsT=(wk[ai // 4][:, ai % 4, m * 128 : (m + 1) * 128]),
                            rhs=(xk_res[tt][0][:, ai, :]),
                            start=(ai == 0),
                            stop=(ai == 7),
                        )
                    nc.vector.tensor_copy(
                        out=kT_sb[:, tt * 512 : (tt + 1) * 512], in_=pp_t
                    )
                if upto < 4:
                    continue
                for h in (2 * m, 2 * m + 1):
                    a_h, off = h // 2, (h % 2) * HD
                    drow = HD if h % 2 == 0 else 0
                    crow = 0 if h % 2 == 0 else HD
                    probs_tiles = []
                    for kt2 in range(NKT // 2):
                        pl_t = pl.tile([128, 1024], F32, tag="pl")
                        for k_i in range(2):
                            kt = 2 * kt2 + k_i
                            nc.tensor.matmul(
                                pl_t[:, k_i * 512 : (k_i + 1) * 512],
                                lhsT=kT_sb[
                                    off : off + HD, kt * 128 : (kt + 1) * 128
                                ],
                                rhs=qT_sb[off : off + HD, a_h, :],
                                start=True,
                                stop=True,
                            )
                        pr = probs_pool.tile([128, 1024], BF16, tag="pr")
                        nc.scalar.activation(
                            out=pr,
                            in_=pl_t,
                            func=mybir.ActivationFunctionType.Exp,
                            scale=1.0 / float(np.sqrt(HD)),
                        )
                        probs_tiles.append(pr)
                    if dbg and h == 0:
                        nc.sync.dma_start(
                            out=dbg_t["probs0"], in_=probs_tiles[0][:, 0:512]
                        )
                    pc_t = pc.tile([128, 512], F32, tag="pc")
                    for kt in range(NKT):
                        nc.tensor.matmul(
                            pc_t,
                            lhsT=_v_head_ap(v_sb, kt, h),
                            rhs=probs_tiles[kt // 2][
                                :, (kt % 2) * 512 : (kt % 2 + 1) * 512
                            ],
                            start=(kt == 0),
                            stop=(kt == NKT - 1),
                        )
                    if dbg and h == 0:
                        pc_cp = small_pool.tile([128, 512], F32, tag="pccp")
                        nc.vector.tensor_copy(out=pc_cp, in_=pc_t)
                        nc.sync.dma_start(out=dbg_t["pc0"], in_=pc_cp)
                    rt = small_pool.tile([128, 512], F32, tag="recip")
                    nc.vector.reciprocal(
                        out=rt[drow : drow + 1, :], in_=pc_t[drow : drow + 1, :]
                    )
                    db = dram_pool.tile([1, 512], F32, tag="db")
                    nc.sync.dma_start(out=db, in_=rt[drow : drow + 1, :])
                    bc = small_pool.tile([128, 512], F32, tag="bc")
                    nc.sync.dma_start(
                        out=bc[crow : crow + HD, :],
                        in_=bass.AP(
                            tensor=db.tensor,
                            offset=db.offset,
                            ap=[[0, HD], [1, 512]],
                        ),
                    )
                    nc.vector.tensor_mul(
                        out=ctxT_sb[off : off + HD, a_h, :],
                        in0=pc_t[crow : crow + HD, :],
                        in1=bc[crow : crow + HD, :],
                    )

            qr_tiles = []
            if upto >= 5:
                for tt in range(SHARD // 128):
                    qr = qres_pool.tile([128, H], F32, tag="qr", name=f"qr_{tt}")
                    nc.sync.dma_start(out=qr, in_=qres_r[:, tt, :])
                    qr_tiles.append(qr)

            if dbg:
                nc.sync.dma_start(out=dbg_t["qT"], in_=qT_sb)
                nc.sync.dma_start(out=dbg_t["v"], in_=v_sb)
                nc.sync.dma_start(out=dbg_t["ctxT"], in_=ctxT_sb.bitcast(F32))

            # ---- P5: output projection + residual + LayerNorm ----
            for tt in range(SHARD // 128 if upto >= 5 else 0):
                qr = qr_tiles[tt]
                resid = out_pool.tile([128, H], F32, tag="resid")
                for ot in range(2):
                    pp_t = pp.tile([128, 512], F32, tag="pp")
                    for ai in range(8):
                        nc.tensor.matmul(
                            pp_t,
                            lhsT=(ctxT_sb[:, ai, tt * 128 : (tt + 1) * 128]),
                            rhs=(wo[ai // 2][:, ai % 2, ot * 512 : (ot + 1) * 512]),
                            start=(ai == 0),
                            stop=(ai == 7),
                        )
                    nc.vector.tensor_add(
                        out=resid[:, ot * 512 : (ot + 1) * 512],
                        in0=pp_t,
                        in1=qr[:, ot * 512 : (ot + 1) * 512],
                    )
                stats = stat_pool.tile([128, 2, 6], F32, tag="stats")
                nc.vector.bn_stats(out=stats[:, 0, :], in_=resid[:, 0:512])
                nc.vector.bn_stats(out=stats[:, 1, :], in_=resid[:, 512:H])
                mv = stat_pool.tile([128, 2], F32, tag="mv")
                nc.vector.bn_aggr(out=mv, in_=stats)
                rstd = stat_pool.tile([128, 1], F32, tag="rstd")
                nc.scalar.activation(
                    out=rstd,
                    in_=mv[:, 1:2],
                    func=mybir.ActivationFunctionType.Sqrt,
                    bias=eps_t,
                    scale=1.0,
                )
                nc.vector.reciprocal(out=rstd, in_=rstd)
                nc.vector.tensor_scalar(
                    out=resid,
                    in0=resid,
                    scalar1=mv[:, 0:1],
                    scalar2=rstd,
                    op0=mybir.AluOpType.subtract,
                    op1=mybir.AluOpType.mult,
                )
                nc.vector.tensor_mul(out=resid, in0=resid, in1=ga_b)
                nc.vector.tensor_add(out=resid, in0=resid, in1=be_b)
                nc.sync.dma_start(out=out[tt * 128 : (tt + 1) * 128, :], in_=resid)

    nc.compile()
    return nc


_NC = None


def _get_nc():
    global _NC
    if _NC is None:
        _NC = build_kernel()
    return _NC


def make_in_maps(query, key, value, Wq, Wk, Wv, Wo, bo, ln_gamma, ln_beta):
    """Host-side sharding: build the per-core input dicts."""
    import ml_dtypes

    bf16 = ml_dtypes.bfloat16
    query = np.asarray(query, np.float32)
    key = np.asarray(key, np.float32)
    value = np.asarray(value, np.float32)
    query16 = query.astype(bf16)
    key16 = key.astype(bf16)
    value16 = value.astype(bf16)
    wqT = np.ascontiguousarray(np.asarray(Wq, np.float32).astype(bf16).T)
    wkT = np.ascontiguousarray(np.asarray(Wk, np.float32).astype(bf16).T)
    wvT = np.ascontiguousarray(np.asarray(Wv, np.float32).astype(bf16).T)
    woT = np.ascontiguousarray(np.asarray(Wo, np.float32).T)
    bo_f = np.asarray(bo, np.float32)
    gbb = np.ascontiguousarray(
        np.stack(
            [np.asarray(ln_gamma, np.float32), np.asarray(ln_beta, np.float32)]
        )
    )
    in_maps = []
    for c in range(N_CORES):
        b = c // (N_CORES // B)
        t0 = (c % (N_CORES // B)) * SHARD
        in_maps.append(
            {
                "xqT": np.ascontiguousarray(query16[b, t0 : t0 + SHARD, :].T),
                "xkT": np.ascontiguousarray(key16[b].T),
                "xvT": np.ascontiguousarray(value16[b].T),
                "wqT": wqT,
                "wkT": wkT,
                "wvT": wvT,
                "woT": woT,
                "qres": np.ascontiguousarray(query[b, t0 : t0 + SHARD, :] + bo_f[None, :]),
                "gbb": gbb,
            }
        )
    return in_maps


def gather_out(results):
    """Concatenate the 8 per-core output shards into [B, S, H]."""
    out = np.empty((B, S, H), np.float32)
    for c in range(N_CORES):
        b = c // (N_CORES // B)
        t0 = (c % (N_CORES // B)) * SHARD
        out[b, t0 : t0 + SHARD, :] = results[c]["out"]
    return out


def kernel(**inputs):
    from concourse.bass_utils import run_bass_kernel_spmd

    nc = _get_nc()
    in_maps = make_in_maps(**inputs)
    res = run_bass_kernel_spmd(nc, in_maps, core_ids=list(range(N_CORES)))
    return gather_out(res.results)


if __name__ == "__main__":
    nc = build_kernel()
    print("built ok")


# revision 32
# speedup vs baseline: 1.2320x; 1.2320x over previous
"""Trainium2 Bass kernel for MultiHeadAttention + residual + LayerNorm.

Problem (fixed shapes):
  B=2, S=2048, H=1024, NH=16, HD=64
  out = LayerNorm(query + (softmax(scale * (query Wq^T)(key Wk^T)^T) (value Wv^T)) Wo^T + bo)

Sharding: 8 cores, data-parallel over (batch, query-token-shard):
  core c -> batch b = c // 4, query rows [ (c%4)*512, (c%4+1)*512 ) of batch b.
Each core recomputes K/V projections for its batch (replicated within the 4
cores of a batch) so no cross-core communication is needed; the host only
slices inputs and concatenates the 8 output shards.

Per-core dataflow (all matmuls on the PE array, fp32r for projections,
bf16 for attention, fp32 accumulation in PSUM):
  P3: qT[j, t]  = Wq^T as lhsT,  xqT as moving     (transposed layout)
  P1: kT[j, t]  = same, over all 2048 keys
  P2: v[t, d]   = xvT as lhsT,   Wv^T as moving    (normal layout), with
      constant-1.0 columns appended on both sides of v so that the ctx
      matmul simultaneously produces the softmax denominator rows.
  P4: per head h: logitsT[k, q] = kT_h^T qT_h ; probsT = exp(scale*logitsT)
      ctxT_aug = [v_h | ones]^T probsT  -> ctx rows + denominator rows
      ctxT_h = ctx rows * (1/denom) broadcast via a K=1 matmul
  P5: projected[t, o] = ctxT as lhsT, Wo^T as moving; + query residual + bo;
      LayerNorm over H via bn_stats/bn_aggr; DMA out.
"""

from contextlib import ExitStack

import numpy as np

import concourse.bass as bass
import concourse.mybir as mybir
import concourse.tile as tile
from concourse import bacc

B, S, H = 2, 2048, 1024
NH, HD = 16, 64
LN_EPS = 1e-5
N_CORES = 8
SHARD = B * S // N_CORES  # 512 query rows per core

F32 = mybir.dt.float32
F32R = mybir.dt.float32r
BF16 = mybir.dt.bfloat16
F16 = mybir.dt.float16

# v tile column layout: 8 head-pair groups of [v_even(64) | ones(64) | v_odd(64)]
# so that every head's [v|ones] (even) / [ones|v] (odd) stationary operand is
# one contiguous 128-column slice (walrus allows only one free dim on matmul APs).
VCOLS = (NH // 2) * 192  # 1536


def _bcast_row(dram_ap_row, parts=128):
    """Partition-broadcast a [1, N] DRAM row to `parts` partitions for DMA."""
    return bass.AP(
        tensor=dram_ap_row.tensor,
        offset=dram_ap_row.offset,
        ap=[[0, parts]] + list(dram_ap_row.ap)[-1:],
    )


def _v_head_ap(v_sb, kt, h):
    """Contiguous 128-col slice [v_h | ones] (even h) / [ones | v_h] (odd h).

    For even h the ones block comes second -> PE output partitions 0-63 hold
    ctx, 64-127 hold the denominator; for odd h it is reversed. This parity
    matches where ctxT_sb wants the rows, keeping vector ops partition-aligned.
    """
    start = (h // 2) * 192 + (64 if h % 2 else 0)
    ap = v_sb[:, kt, start : start + 128]
    assert ap.free_size() == 128 and ap.partition_size() == 128
    return ap


def build_kernel(dbg: bool = False, upto: int = 5, reps: int = 1, xbufs: int = 5, prbufs: int = 3, wbufs: int = 4):
    nc = bacc.Bacc("TRN2", target_bir_lowering=False, debug=False)

    xqT = nc.dram_tensor("xqT", [H, SHARD], BF16, kind="ExternalInput").ap()
    xkT = nc.dram_tensor("xkT", [H, S], BF16, kind="ExternalInput").ap()
    xvT = nc.dram_tensor("xvT", [H, S], BF16, kind="ExternalInput").ap()
    wqT = nc.dram_tensor("wqT", [H, H], BF16, kind="ExternalInput").ap()
    wkT = nc.dram_tensor("wkT", [H, H], BF16, kind="ExternalInput").ap()
    wvT = nc.dram_tensor("wvT", [H, H], BF16, kind="ExternalInput").ap()
    woT = nc.dram_tensor("woT", [H, H], F32R, kind="ExternalInput").ap()
    qres = nc.dram_tensor("qres", [SHARD, H], F32, kind="ExternalInput").ap()
    gbb = nc.dram_tensor("gbb", [2, H], F32, kind="ExternalInput").ap()
    out = nc.dram_tensor("out", [SHARD, H], F32, kind="ExternalOutput").ap()
    dbg_t = {}
    if dbg:
        dbg_t["qT"] = nc.dram_tensor("dbg_qT", [128, H // 128, SHARD], BF16, kind="ExternalOutput").ap()
        dbg_t["v"] = nc.dram_tensor("dbg_v", [128, S // 128, VCOLS], BF16, kind="ExternalOutput").ap()
        dbg_t["ctxT"] = nc.dram_tensor("dbg_ctxT", [128, H // 128, SHARD], F32, kind="ExternalOutput").ap()
        dbg_t["probs0"] = nc.dram_tensor("dbg_probs0", [128, 512], BF16, kind="ExternalOutput").ap()
        dbg_t["pc0"] = nc.dram_tensor("dbg_pc0", [128, 512], F32, kind="ExternalOutput").ap()

    # [feature-tile p, feature-group a, token t] views for DMA
    xqT_r = xqT.rearrange("(a p) t -> p a t", p=128)
    xkT_r = xkT.rearrange("(a p) t -> p a t", p=128)
    xvT_r = xvT.rearrange("(a p) t -> p a t", p=128)
    w_r = {
        "wq": wqT.rearrange("(a p) j -> p a j", p=128),
        "wk": wkT.rearrange("(a p) j -> p a j", p=128),
        "wv": wvT.rearrange("(a p) j -> p a j", p=128),
        "wo": woT.rearrange("(a p) j -> p a j", p=128),
    }
    qres_r = qres.rearrange("(a p) j -> p a j", p=128)

    KCH = S // 512  # 4 key-token chunks
    QCH = SHARD // 512  # 1 query chunk

    with ExitStack() as ctx:
        ctx.enter_context(nc.allow_low_precision(reason="bf16 attention + f32r matmuls by design"))
        tc = ctx.enter_context(tile.TileContext(nc))
        wpool = ctx.enter_context(tc.tile_pool(name="wpool", bufs=wbufs))
        xpool = ctx.enter_context(tc.tile_pool(name="xpool", bufs=xbufs))
        kT_pool = ctx.enter_context(tc.tile_pool(name="kT", bufs=3))
        v_pool = ctx.enter_context(tc.tile_pool(name="vv", bufs=1))
        qT_pool = ctx.enter_context(tc.tile_pool(name="qT", bufs=1))
        ctx_pool = ctx.enter_context(tc.tile_pool(name="ctxT", bufs=1))
        probs_pool = ctx.enter_context(tc.tile_pool(name="probs", bufs=prbufs))
        small_pool = ctx.enter_context(tc.tile_pool(name="smalls", bufs=2))
        const_pool = ctx.enter_context(tc.tile_pool(name="consts", bufs=1))
        qres_pool = ctx.enter_context(tc.tile_pool(name="qresp", bufs=2))
        out_pool = ctx.enter_context(tc.tile_pool(name="outp", bufs=2))
        stat_pool = ctx.enter_context(tc.tile_pool(name="stats", bufs=2))
        dram_pool = ctx.enter_context(tc.tile_pool(name="drp", bufs=4, space="DRAM"))
        pp = ctx.enter_context(tc.tile_pool(name="pp", bufs=2, space="PSUM"))
        pl = ctx.enter_context(tc.tile_pool(name="pl", bufs=2, space="PSUM"))
        pc = ctx.enter_context(tc.tile_pool(name="pc", bufs=2, space="PSUM"))
        rep_ctx = (
            tc.For_i(
                0,
                reps,
                1,
                hint_engines=(
                    mybir.EngineType.PE,
                    mybir.EngineType.Activation,
                    mybir.EngineType.DVE,
                    mybir.EngineType.SP,
                    mybir.EngineType.Pool,
                ),
            )
            if reps > 1
            else None
        )
        if rep_ctx is not None:
            ctx.enter_context(rep_ctx)
        if True:
            # ---- constants ----
            eps_t = const_pool.tile([128, 1], F32)
            nc.vector.memset(eps_t, LN_EPS)
            ga_b = const_pool.tile([128, H], F32)
            be_b = const_pool.tile([128, H], F32)
            for i, t in enumerate((ga_b, be_b)):
                nc.sync.dma_start(out=t, in_=_bcast_row(gbb[i : i + 1, :]))

            # ---- persistent activations ----
            v_sb = v_pool.tile([128, S // 128, VCOLS], BF16)  # v[t, d] + ones
            qT_sb = qT_pool.tile([128, H // 128, SHARD], BF16)  # qT[j, t]
            ctxT_sb = ctx_pool.tile([128, H // 128, SHARD], F32R)  # ctxT[i, t]

            def load_w_halves(name):
                # two bf16 tiles of 4 feature-groups each (8KB/partition slots)
                h0 = wpool.tile([128, 4, H], BF16, tag="w")
                h1 = wpool.tile([128, 4, H], BF16, tag="w")
                nc.sync.dma_start(out=h0, in_=w_r[name][:, 0:4, :])
                nc.sync.dma_start(out=h1, in_=w_r[name][:, 4:8, :])
                return (h0, h1)

            def load_w_quarters(name):
                # four f32r tiles of 2 feature-groups each (8KB/partition slots)
                qs = []
                for i in range(4):
                    q = wpool.tile([128, 2, H], F32R, tag="w")
                    nc.sync.dma_start(out=q, in_=w_r[name][:, 2 * i : 2 * i + 2, :])
                    qs.append(q)
                return qs

            def load_x_chunk(x_r, tt):
                t = xpool.tile([128, 8, 512], BF16, tag="x")
                nc.sync.dma_start(out=t, in_=x_r[:, :, tt * 512 : (tt + 1) * 512])
                return (t, t)

            # ---- P3: q projection (transposed layout, upfront) ----
            wq = load_w_halves("wq")
            for tt in range(QCH):
                xq = load_x_chunk(xqT_r, tt)
                for aj in range(8):
                    pp_t = pp.tile([128, 512], F32, tag="pp")
                    for ai in range(8):
                        nc.tensor.matmul(
                            pp_t,
                            lhsT=(wq[ai // 4][:, ai % 4, aj * 128 : (aj + 1) * 128]),
                            rhs=(xq[0][:, ai, :]),
                            start=(ai == 0),
                            stop=(ai == 7),
                        )
                    nc.vector.tensor_copy(
                        out=qT_sb[:, aj, tt * 512 : (tt + 1) * 512], in_=pp_t
                    )

            # ---- P2: v projection (normal layout, with ones columns) ----
            wv = load_w_halves("wv")
            for tt in range(KCH):
                xv = load_x_chunk(xvT_r, tt)
                for tloc in range(4):
                    at = tt * 4 + tloc
                    base = v_sb[:, at, :]
                    ones_ap = bass.AP(
                        tensor=base.tensor,
                        offset=base.offset + 64,
                        ap=[list(base.ap)[0], [192, NH // 2], [1, 64]],
                    )
                    nc.vector.memset(ones_ap, 1.0)
                    trow = slice(tloc * 128, (tloc + 1) * 128)
                    for dt in range(2):
                        pp_t = pp.tile([128, 512], F32, tag="pp")
                        for ai in range(8):
                            nc.tensor.matmul(
                                pp_t,
                                lhsT=(xv[0][:, ai, trow]),
                                rhs=(
                                    wv[ai // 4][:, ai % 4, dt * 512 : (dt + 1) * 512]
                                ),
                                start=(ai == 0),
                                stop=(ai == 7),
                            )
                        v_dst = bass.AP(
                            tensor=base.tensor,
                            offset=base.offset + dt * 4 * 192,
                            ap=[list(base.ap)[0], [192, 4], [128, 2], [1, 64]],
                        )
                        nc.vector.tensor_copy(out=v_dst, in_=pp_t)

            # ---- prefetch output-projection weight + residual ----
            wk = load_w_halves("wk")
            wo = load_w_quarters("wo") if upto >= 5 else None

            # ---- P1+P4 interleaved per head pair ----
            # xk stays resident (4 chunks); kproj for feature block m (heads
            # 2m, 2m+1) writes a small rotating kT tile, then both heads'
            # logits/exp/ctx run while the next pair's kproj proceeds -> the
            # ACT exp stream starts ~1/8 into the k projection.
            NKT = S // 128
            xk_res = [load_x_chunk(xkT_r, tt) for tt in range(KCH)]
            for m in range(NH // 2):
                kT_sb = kT_pool.tile([128, S], BF16, tag="kt", name=f"kT_{m}")
                for tt in range(KCH):
                    pp_t = pp.tile([128, 512], F32, tag="pp")
                    for ai in range(8):
                        nc.tensor.matmul(
                            pp_t,
                            lhsT=(wk[ai // 4][:, ai % 4, m * 128 : (m + 1) * 128]),
                            rhs=(xk_res[tt][0][:, ai, :]),
                            start=(ai == 0),
                            stop=(ai == 7),
                        )
                    nc.vector.tensor_copy(
                        out=kT_sb[:, tt * 512 : (tt + 1) * 512], in_=pp_t
                    )
                if upto < 4:
                    continue
                for h in (2 * m, 2 * m + 1):
                    a_h, off = h // 2, (h % 2) * HD
                    drow = HD if h % 2 == 0 else 0
                    crow = 0 if h % 2 == 0 else HD
                    probs_tiles = []
                    for kt2 in range(NKT // 2):
                        pl_t = pl.tile([128, 1024], F32, tag="pl")
                        for k_i in range(2):
                            kt = 2 * kt2 + k_i
                            nc.tensor.matmul(
                                pl_t[:, k_i * 512 : (k_i + 1) * 512],
                                lhsT=kT_sb[
                                    off : off + HD, kt * 128 : (kt + 1) * 128
                                ],
                                rhs=qT_sb[off : off + HD, a_h, :],
                                start=True,
                                stop=True,
                            )
                        pr = probs_pool.tile([128, 1024], BF16, tag="pr")
                        nc.scalar.activation(
                            out=pr,
                            in_=pl_t,
                            func=mybir.ActivationFunctionType.Exp,
                            scale=1.0 / float(np.sqrt(HD)),
                        )
                        probs_tiles.append(pr)
                    if dbg and h == 0:
                        nc.sync.dma_start(
                            out=dbg_t["probs0"], in_=probs_tiles[0][:, 0:512]
                        )
                    pc_t = pc.tile([128, 512], F32, tag="pc")
                    for kt in range(NKT):
                        nc.tensor.matmul(
                            pc_t,
                            lhsT=_v_head_ap(v_sb, kt, h),
                            rhs=probs_tiles[kt // 2][
                                :, (kt % 2) * 512 : (kt % 2 + 1) * 512
                            ],
                            start=(kt == 0),
                            stop=(kt == NKT - 1),
                        )
                    if dbg and h == 0:
                        pc_cp = small_pool.tile([128, 512], F32, tag="pccp")
                        nc.vector.tensor_copy(out=pc_cp, in_=pc_t)
                        nc.sync.dma_start(out=dbg_t["pc0"], in_=pc_cp)
                    rt = small_pool.tile([128, 512], F32, tag="recip")
                    nc.vector.reciprocal(
                        out=rt[drow : drow + 1, :], in_=pc_t[drow : drow + 1, :]
                    )
                    db = dram_pool.tile([1, 512], F32, tag="db")
                    nc.sync.dma_start(out=db, in_=rt[drow : drow + 1, :])
                    bc = small_pool.tile([128, 512], F32, tag="bc")
                    nc.sync.dma_start(
                        out=bc[crow : crow + HD, :],
                        in_=bass.AP(
                            tensor=db.tensor,
                            offset=db.offset,
                            ap=[[0, HD], [1, 512]],
                        ),
                    )
                    nc.vector.tensor_mul(
                        out=ctxT_sb[off : off + HD, a_h, :],
                        in0=pc_t[crow : crow + HD, :],
                        in1=bc[crow : crow + HD, :],
                    )

            qr_tiles = []
            if upto >= 5:
                for tt in range(SHARD // 128):
                    qr = qres_pool.tile([128, H], F32, tag="qr", name=f"qr_{tt}")
                    nc.sync.dma_start(out=qr, in_=qres_r[:, tt, :])
                    qr_tiles.append(qr)

            if dbg:
                nc.sync.dma_start(out=dbg_t["qT"], in_=qT_sb)
                nc.sync.dma_start(out=dbg_t["v"], in_=v_sb)
                nc.sync.dma_start(out=dbg_t["ctxT"], in_=ctxT_sb.bitcast(F32))

            # ---- P5: output projection + residual + LayerNorm ----
            for tt in range(SHARD // 128 if upto >= 5 else 0):
                qr = qr_tiles[tt]
                resid = out_pool.tile([128, H], F32, tag="resid")
                for ot in range(2):
                    pp_t = pp.tile([128, 512], F32, tag="pp")
                    for ai in range(8):
                        nc.tensor.matmul(
                            pp_t,
                            lhsT=(ctxT_sb[:, ai, tt * 128 : (tt + 1) * 128]),
                            rhs=(wo[ai // 2][:, ai % 2, ot * 512 : (ot + 1) * 512]),
                            start=(ai == 0),
                            stop=(ai == 7),
                        )
                    nc.vector.tensor_add(
                        out=resid[:, ot * 512 : (ot + 1) * 512],
                        in0=pp_t,
                        in1=qr[:, ot * 512 : (ot + 1) * 512],
                    )
                stats = stat_pool.tile([128, 2, 6], F32, tag="stats")
                nc.vector.bn_stats(out=stats[:, 0, :], in_=resid[:, 0:512])
                nc.vector.bn_stats(out=stats[:, 1, :], in_=resid[:, 512:H])
                mv = stat_pool.tile([128, 2], F32, tag="mv")
                nc.vector.bn_aggr(out=mv, in_=stats)
                rstd = stat_pool.tile([128, 1], F32, tag="rstd")
                nc.scalar.activation(
                    out=rstd,
                    in_=mv[:, 1:2],
                    func=mybir.ActivationFunctionType.Sqrt,
                    bias=eps_t,
                    scale=1.0,
                )
                nc.vector.reciprocal(out=rstd, in_=rstd)
                nc.vector.tensor_scalar(
                    out=resid,
                    in0=resid,
                    scalar1=mv[:, 0:1],
                    scalar2=rstd,
                    op0=mybir.AluOpType.subtract,
                    op1=mybir.AluOpType.mult,
                )
                nc.vector.tensor_mul(out=resid, in0=resid, in1=ga_b)
                nc.vector.tensor_add(out=resid, in0=resid, in1=be_b)
                nc.sync.dma_start(out=out[tt * 128 : (tt + 1) * 128, :], in_=resid)

    nc.compile()
    return nc


_NC = None


def _get_nc():
    global _NC
    if _NC is None:
        _NC = build_kernel()
    return _NC


def make_in_maps(query, key, value, Wq, Wk, Wv, Wo, bo, ln_gamma, ln_beta):
    """Host-side sharding: build the per-core input dicts."""
    import ml_dtypes

    bf16 = ml_dtypes.bfloat16
    query = np.asarray(query, np.float32)
    key = np.asarray(key, np.float32)
    value = np.asarray(value, np.float32)
    query16 = query.astype(bf16)
    key16 = key.astype(bf16)
    value16 = value.astype(bf16)
    wqT = np.ascontiguousarray(np.asarray(Wq, np.float32).astype(bf16).T)
    wkT = np.ascontiguousarray(np.asarray(Wk, np.float32).astype(bf16).T)
    wvT = np.ascontiguousarray(np.asarray(Wv, np.float32).astype(bf16).T)
    woT = np.ascontiguousarray(np.asarray(Wo, np.float32).T)
    bo_f = np.asarray(bo, np.float32)
    gbb = np.ascontiguousarray(
        np.stack(
            [np.asarray(ln_gamma, np.float32), np.asarray(ln_beta, np.float32)]
        )
    )
    in_maps = []
    for c in range(N_CORES):
        b = c // (N_CORES // B)
        t0 = (c % (N_CORES // B)) * SHARD
        in_maps.append(
            {
                "xqT": np.ascontiguousarray(query16[b, t0 : t0 + SHARD, :].T),
                "xkT": np.ascontiguousarray(key16[b].T),
                "xvT": np.ascontiguousarray(value16[b].T),
                "wqT": wqT,
                "wkT": wkT,
                "wvT": wvT,
                "woT": woT,
                "qres": np.ascontiguousarray(query[b, t0 : t0 + SHARD, :] + bo_f[None, :]),
                "gbb": gbb,
            }
        )
    return in_maps


def gather_out(results):
    """Concatenate the 8 per-core output shards into [B, S, H]."""
    out = np.empty((B, S, H), np.float32)
    for c in range(N_CORES):
        b = c // (N_CORES // B)
        t0 = (c % (N_CORES // B)) * SHARD
        out[b, t0 : t0 + SHARD, :] = results[c]["out"]
    return out


def kernel(**inputs):
    from concourse.bass_utils import run_bass_kernel_spmd

    nc = _get_nc()
    in_maps = make_in_maps(**inputs)
    res = run_bass_kernel_spmd(nc, in_maps, core_ids=list(range(N_CORES)))
    return gather_out(res.results)


if __name__ == "__main__":
    nc = build_kernel()
    print("built ok")
